# revision 58
# baseline (speedup 1.0000x reference)
"""Trainium2 Bass kernel for GPT2-style single attention layer.

Problem: B=4, S=2048, E=1024, H=16 heads, D=64.
  x = hidden @ W_attn + b_attn ; q,k,v = split(x)
  per head: softmax(causal(q k^T / 8) + mask) @ v
  out = merge @ W_proj + b_proj

Sharding over 8 cores: core i -> batch b = i//2, heads h0 = (i%2)*8 .. +8
(data parallel on B, tensor parallel over heads).  Each core's work is fully
local; the host sums the two partial projections per batch.

Dataflow is fully "transposed" so no on-chip transposes are ever needed:
  host feeds xT = hidden[b].T                       [E, S]
  Q^T,K^T = (Wq|Wk block)^T @ xT     -> [d, tok] per head   (W stationary)
  V       = xT_block^T @ Wv          -> [tok, d] natural    (xT stationary)
  S^T     = K^T_blk^T @ Q^T          -> [k, q]   (softmax dim on partitions)
  P^T     = exp(0.125*S^T + mask[k]) * causal01
  sums    = ones^T @ P^T             -> [1, q]  (ones-matmul, PSUM-accum)
  attn^T  = V_blk^T @ P^T            -> [d, q]  accumulated over k tiles
  norm    = attn^T * broadcast(1/sums)   (broadcast via K=1 ones-matmul)
  out^T   = Wp_blk^T @ attn^T        -> [col, tok]
Host transposes out^T back and sums core pairs + b_proj.

All matmuls run as bf16 (full-rate fp32 path, 1 cycle/row at N>=256).
The six small constants are packed into two tensors and loaded on the
Activation DGE ring so the critical SP ring only streams the big inputs.

Startup: ~20 dummy matmuls on a locally-memset tile run from ~0.3us with no
DMA dependency, keeping the PE HAM clock-gate warm while the first input
DMAs land; wv is loaded whole and x quarter 0 in column chunks matched to
v_tt's consumption order, so the first real matmul fires ~7us earlier than
the half-interleaved load order did.
"""

import os
import ml_dtypes
import numpy as np

B, S, E, H, D = 4, 2048, 1024, 16, 64
NC = 8
HL = H // 2          # local heads per core
EL = HL * D          # local embedding slice = 512
P = 128              # partitions
QT = 512             # q tile width (f32 moving max)
NQT = S // QT        # 4 q tiles
NKT = S // P         # 16 k tiles
NET = E // P         # 8 e (contraction) tiles

_CACHE = {}
LAST_RESULT = None


def _build(has_bv: bool):
    from contextlib import ExitStack

    import concourse.tile as tile
    from concourse import bacc, mybir

    f32 = mybir.dt.float32
    f32r = mybir.dt.bfloat16  # matmul operand dtype (2-byte: full-rate moving operand)
    EXP = mybir.ActivationFunctionType.Exp

    nc = bacc.Bacc(
        "TRN2",
        target_bir_lowering=False,
        debug=False,
        enable_asserts=False,
        num_devices=NC,
    )

    def inp(name, shape, dt=f32):
        return nc.dram_tensor(name, shape, dt, kind="ExternalInput").ap()

    xt_d = inp("xt", [E, S], f32r)
    wq_d = inp("wq", [E, EL], f32r)
    wk_d = inp("wk", [E, EL], f32r)
    wv_d = inp("wv", [E, EL], f32r)
    wp_d = inp("wp", [EL, E], f32r)
    constf_d = inp("constf", [P, 28])          # bq | bk | bv | maskt
    constr_d = inp("constr", [P, 192], f32r)   # causal | ones
    out_d = nc.dram_tensor("out", [E, S], f32r, kind="ExternalOutput").ap()


    with tile.TileContext(nc) as tc, ExitStack() as ctx:
        const = ctx.enter_context(tc.tile_pool(name="const", bufs=1))
        big = ctx.enter_context(tc.tile_pool(name="big", bufs=1))
        wpool = ctx.enter_context(tc.tile_pool(name="wpool", bufs=1))
        xpool = ctx.enter_context(tc.tile_pool(name="xpool", bufs=1))
        ptpool = ctx.enter_context(tc.tile_pool(name="ptpool", bufs=1))
        aopool = ctx.enter_context(tc.tile_pool(name="aopool", bufs=1))
        ospool = ctx.enter_context(tc.tile_pool(name="ospool", bufs=1))
        rcpool = ctx.enter_context(tc.tile_pool(name="rcpool", bufs=1))
        aospool = ctx.enter_context(tc.tile_pool(name="aospool", bufs=1))
        psum = ctx.enter_context(tc.tile_pool(name="psum", bufs=1, space="PSUM"))


        # ---- PE warm-up: dummy matmuls with no DMA deps keep the HAM
        # clock-gate un-throttled while the first input DMAs are in flight,
        # so the first real matmuls run at 2.4 GHz instead of 1.2.
        warm = const.tile([P, QT], f32r, name="warm")
        nc.vector.memset(warm[:], 0.5)
        wst = psum.tile([P, 2 * QT], f32, name="wst", tag="st", bufs=2)
        for _ in range(20):
            nc.tensor.matmul(wst[:, 0:QT], warm[:, 0:P], warm[:],
                             start=True, stop=True)

        # ---- persistent big buffers ----
        # Q^T / K^T: per head-pair p a [128, S] tile (partitions = 2 heads x 64 d)
        qt_tiles = [big.tile([P, S], f32r, name=f"qt{p}", tag=f"qt{p}") for p in range(4)]
        kt_tiles = [big.tile([P, S], f32r, name=f"kt{p}", tag=f"kt{p}") for p in range(4)]
        # V natural: 16 tiles [128 tok, 512 vcol]
        v_tiles = [big.tile([P, 8 * 65], f32r, name=f"v{t}", tag=f"v{t}") for t in range(NKT)]

        # the two packed const tensors ride the Activation DGE ring so the
        # SP ring streams only the big inputs (first exp gates ~3us earlier)
        constf_t = const.tile([P, 28], f32, name="constf_t")
        nc.scalar.dma_start(constf_t[:], constf_d[:])
        constr_t = const.tile([P, 192], f32r, name="constr_t")
        nc.scalar.dma_start(constr_t[:], constr_d[:])
        bq_t = constf_t[:, 0:4]
        bk_t = constf_t[:, 4:8]
        bv_t = constf_t[:, 8:12]
        maskt_t = constf_t[:, 12:28]
        causal_t = constr_t[:, 0:128]
        ones_t = constr_t[:, 128:192]

        # DMA emission ordered by first use: wv + xT quarter 0 gate the first
        # compute; the rest stream in behind.
        x_tiles = [[None] * NQT for _ in range(NET)]

        def load_x_quarter(tq):
            xb = xpool.tile([P, NET * QT], f32r, name=f"xb{tq}", tag=f"xb{tq}", bufs=1)
            nc.sync.dma_start(
                xb[:].rearrange("p (a c) -> p a c", a=NET, c=QT),
                xt_d.rearrange("(a p) s -> p a s", p=P)[:, :, tq * QT:(tq + 1) * QT],
            )
            for kt in range(NET):
                x_tiles[kt][tq] = xb[:, kt * QT:(kt + 1) * QT]

        def load_w_big(dram, label):
            wb = wpool.tile([P, NET * EL], f32r, name=f"wb_{label}", tag=f"wb_{label}",
                            bufs=1)
            nc.sync.dma_start(
                wb[:].rearrange("p (a c) -> p a c", a=NET, c=EL),
                dram.rearrange("(a p) c -> p a c", p=P),
            )
            return [wb[:, kt * EL:(kt + 1) * EL] for kt in range(NET)]

        # startup: wv whole, then x quarter 0 in column chunks matched to
        # v_tt's consumption order (tt=0 needs cols 0:128 of every e-block),
        # so the first V accumulation group starts as soon as they land
        wvb = wpool.tile([P, NET * EL], f32r, name="wb_v", tag="wb_v", bufs=1)
        nc.sync.dma_start(
            wvb[:].rearrange("p (a c) -> p a c", a=NET, c=EL),
            wv_d.rearrange("(a p) c -> p a c", p=P),
        )
        wv_t = [wvb[:, kt * EL:(kt + 1) * EL] for kt in range(NET)]
        xb0 = xpool.tile([P, NET * QT], f32r, name="xb0", tag="xb0", bufs=1)
        xv0 = xb0[:].rearrange("p (a c) -> p a c", a=NET, c=QT)
        xs0 = xt_d.rearrange("(a p) s -> p a s", p=P)
        nc.sync.dma_start(xv0[:, :, 0:P], xs0[:, :, 0:P])
        nc.sync.dma_start(xv0[:, :, P:QT], xs0[:, :, P:QT])
        for kt in range(NET):
            x_tiles[kt][0] = xb0[:, kt * QT:(kt + 1) * QT]
        wq_t = load_w_big(wq_d, "q")
        wk_t = load_w_big(wk_d, "k")
        load_x_quarter(1)
        wpb = wpool.tile([P, 4 * E], f32r, name="wpb", tag="wpb", bufs=1)
        nc.sync.dma_start(
            wpb[:].rearrange("p (a c) -> p a c", a=4, c=E),
            wp_d.rearrange("(a p) c -> p a c", p=P),
        )
        wp_tiles = [wpb[:, p * E:(p + 1) * E] for p in range(4)]
        for tq in range(2, NQT):
            load_x_quarter(tq)



        # ---- per-group compute units (run directly or as PE fillers) ----
        done = set()

        def v_tt(tq, tt):
            key = ("v", tq, tt)
            if key in done:
                return
            done.add(key)
            ps = psum.tile([P, EL], f32, name=f"psv{tq}_{tt}", tag="mm", bufs=2)
            for kt in range(NET):
                nc.tensor.matmul(
                    ps[:], x_tiles[kt][tq][:, tt * P:(tt + 1) * P], wv_t[kt][:],
                    start=(kt == 0), stop=(kt == NET - 1))
            vt = v_tiles[tq * 4 + tt]
            v8 = vt[:, 0:520].rearrange("p (a c) -> p a c", a=8, c=65)
            nc.vector.tensor_copy(
                v8[:, :, 0:64], ps[:].rearrange("p (a c) -> p a c", a=8, c=64))
            nc.gpsimd.memset(v8[:, :, 64:65], 1.0)

        def q_ct(tq, ct):
            key = ("q", tq, ct)
            if key in done:
                return
            done.add(key)
            ps = psum.tile([P, QT], f32, name=f"psq{tq}_{ct}", tag="mm", bufs=2)
            for kt in range(NET):
                nc.tensor.matmul(ps[:], wq_t[kt][:, ct * P:(ct + 1) * P],
                                 x_tiles[kt][tq][:],
                                 start=(kt == 0), stop=(kt == NET - 1))
            nc.vector.tensor_scalar_add(
                qt_tiles[ct][:, tq * QT:(tq + 1) * QT], ps[:], bq_t[:, ct:ct + 1])

        def k_ct(tq, ct):
            key = ("k", tq, ct)
            if key in done:
                return
            done.add(key)
            ps = psum.tile([P, QT], f32, name=f"psk{tq}_{ct}", tag="mm", bufs=2)
            for kt in range(NET):
                nc.tensor.matmul(ps[:], wk_t[kt][:, ct * P:(ct + 1) * P],
                                 x_tiles[kt][tq][:],
                                 start=(kt == 0), stop=(kt == NET - 1))
            nc.vector.tensor_scalar_add(
                kt_tiles[ct][:, tq * QT:(tq + 1) * QT], ps[:], bk_t[:, ct:ct + 1])

        def proj_ct(qt, ct, ao_tiles, tail=False):
            key = ("p", qt, ct)
            if key in done:
                return
            done.add(key)
            ps = psum.tile([P, QT], f32, name=f"psp{qt}_{ct}", tag="mm", bufs=2)
            for p in range(4):
                nc.tensor.matmul(ps[:], wp_tiles[p][:, ct * P:(ct + 1) * P],
                                 ao_tiles[p][:], start=(p == 0), stop=(p == 3))
            osb = ospool.tile([P, QT], f32r, name=f"os{qt}_{ct}", tag="os", bufs=2)
            nc.vector.tensor_copy(osb[:], ps[:])
            nc.sync.dma_start(out_d[ct * P:(ct + 1) * P, qt * QT:(qt + 1) * QT],
                              osb[:])

        fillers = []
        late_fillers = []   # reserved for the last quarter's ACT-bound stretch
        held_tail = []      # reserved for the tail, under the recip chain

        def drain_filler(allow_late=False):
            while fillers:
                fn = fillers.pop(0)
                if fn():  # returns True if it actually emitted work
                    return
            if allow_late:
                while late_fillers:
                    fn = late_fillers.pop(0)
                    if fn():
                        return


        def attention(p, qt, sga, sgb, allow_late=False):
            """Head pair p (heads 2p, 2p+1), q tile qt.

            Leaves attnout halves in an SBUF tile (f32) and the softmax
            denominators in rows 32*p of sga/sgb.  Normalization happens
            batched per qt in normalize()."""
            kt_max = 4 * (qt + 1)
            qsl = slice(qt * QT, (qt + 1) * QT)
            # row 64 of each av accumulates the softmax denominator (ones col)
            ava = psum.tile([65, QT], f32, name=f"ava{p}_{qt}", tag="ava", bufs=1)
            avb = psum.tile([65, QT], f32, name=f"avb{p}_{qt}", tag="avb", bufs=1)

            def av_sums(kt, pt, off):
                first, last = kt == 0, kt == kt_max - 1
                vva = v_tiles[kt][:, (2 * p) * 65:(2 * p + 1) * 65]
                vvb = v_tiles[kt][:, (2 * p + 1) * 65:(2 * p + 2) * 65]
                nc.tensor.matmul(ava[:, off:QT], vva, pt[:, off:QT],
                                 start=first, stop=last)
                nc.tensor.matmul(avb[:, off:QT], vvb, pt[:, QT + off:2 * QT],
                                 start=first, stop=last)

            pending = None
            for kt in range(kt_max):
                # diagonal tiles: only q columns >= off are unmasked
                diag = kt >= qt * 4
                off = (kt - qt * 4) * P if diag else 0
                kl = slice(kt * P, (kt + 1) * P)
                qv = slice(qt * QT + off, (qt + 1) * QT)
                st = psum.tile([P, 2 * QT], f32, name=f"st{p}_{qt}_{kt}",
                               tag="st", bufs=2)
                nc.tensor.matmul(st[:, off:QT], kt_tiles[p][0:64, kl],
                                 qt_tiles[p][0:64, qv])
                nc.tensor.matmul(st[:, QT + off:2 * QT], kt_tiles[p][64:128, kl],
                                 qt_tiles[p][64:128, qv])
                pt = ptpool.tile([P, 2 * QT], f32r, name=f"pt{p}_{qt}_{kt}",
                                 tag="pt", bufs=5)
                bias = maskt_t[:, kt:kt + 1]
                if not diag or off == 0:
                    nc.scalar.activation(pt[:], st[:], EXP, bias=bias, scale=0.125)
                else:
                    stv = st[:].rearrange("p (h q) -> p h q", h=2, q=QT)[:, :, off:QT]
                    ptv = pt[:].rearrange("p (h q) -> p h q", h=2, q=QT)[:, :, off:QT]
                    nc.scalar.activation(ptv, stv, EXP, bias=bias, scale=0.125)
                if diag:
                    # triangular band at the leading 128 valid columns
                    nc.vector.tensor_mul(pt[:, off:off + P], pt[:, off:off + P],
                                         causal_t[:])
                    nc.vector.tensor_mul(pt[:, QT + off:QT + off + P],
                                         pt[:, QT + off:QT + off + P], causal_t[:])
                if pending is not None:
                    av_sums(*pending)
                    if kt % 2 == 0:
                        drain_filler(allow_late)
                pending = (kt, pt, off)
            av_sums(*pending)

            # drain PSUM immediately so the next pair's AV can start
            aos = aospool.tile([P, QT], f32r, name=f"aos{p}_{qt}",
                               tag=f"aos{p}", bufs=2)
            nc.vector.tensor_copy(aos[0:64, :], ava[0:64, :])
            nc.vector.tensor_copy(aos[64:128, :], avb[0:64, :])
            row = 32 * p
            nc.vector.tensor_copy(sga[row:row + 1, :], ava[64:65, :])
            nc.vector.tensor_copy(sgb[row:row + 1, :], avb[64:65, :])
            return aos

        def normalize_pre(qt, sga, sgb):
            """DVE half of the softmax normalization: 1/sums in bf16."""
            rcf = rcpool.tile([97, QT], f32, name=f"rcf{qt}", tag="rcf", bufs=1)
            rcg = rcpool.tile([97, QT], f32, name=f"rcg{qt}", tag="rcg", bufs=1)
            nc.vector.reciprocal_approx_fast(rcf[:], sga[:])
            nc.vector.reciprocal_approx_fast(rcg[:], sgb[:])
            rca = rcpool.tile([97, QT], f32r, name=f"rca{qt}", tag="rca", bufs=1)
            rcb = rcpool.tile([97, QT], f32r, name=f"rcb{qt}", tag="rcb", bufs=1)
            nc.vector.tensor_copy(rca[:], rcf[:])
            nc.vector.tensor_copy(rcb[:], rcg[:])
            return rca, rcb

        def normalize_post(qt, rca, rcb, aos_tiles):
            """PE/DVE half: broadcast 1/sums and scale the attention output."""
            ao_tiles = []
            for p in range(4):
                row = 32 * p
                ao = aopool.tile([P, QT], f32r, name=f"ao{p}_{qt}",
                                 tag=f"ao{p}", bufs=2)
                for half, rcx in ((0, rca), (1, rcb)):
                    rb = psum.tile([64, QT], f32, name=f"rb{p}_{qt}_{half}",
                                   tag="mm", bufs=2)
                    nc.tensor.matmul(rb[:], ones_t[row:row + 1, 0:64],
                                     rcx[row:row + 1, :], tile_position=(row, 0))
                    nc.vector.tensor_mul(ao[64 * half:64 * (half + 1), :], rb[:],
                                         aos_tiles[p][64 * half:64 * (half + 1), :])
                if has_bv:
                    nc.vector.tensor_scalar_add(ao[:], ao[:], bv_t[:, p:p + 1])
                ao_tiles.append(ao)
            return ao_tiles

        def normalize(qt, sga, sgb, aos_tiles):
            """Batched softmax normalization for all 4 pairs of one q tile."""
            rca, rcb = normalize_pre(qt, sga, sgb)
            return normalize_post(qt, rca, rcb, aos_tiles)


        # ============ filler-queue main schedule ============
        # Attention k-loops are ACT(exp)-paced; PE idle slots are filled with
        # independent matmul groups: next quarter's V/Q/K and deferred proj.
        def mkfiller(fn, *args):
            def run():
                before = len(done)
                fn(*args)
                return len(done) != before
            return run

        pending_np = None
        for tq in range(NQT):
            # mandatory prelude: V + first pair's Q/K; later pairs become
            # fillers drained (or ensured) just in time
            for tt in range(4):
                v_tt(tq, tt)
            q_ct(tq, 0)
            k_ct(tq, 0)
            for ct in range(1, 4):
                fillers.append(mkfiller(q_ct, tq, ct))
                fillers.append(mkfiller(k_ct, tq, ct))
            # queue next quarter's V/Q/K as fillers
            if tq + 1 < NQT:
                for tt in range(4):
                    fillers.append(mkfiller(v_tt, tq + 1, tt))
                for ct in range(4):
                    fillers.append(mkfiller(q_ct, tq + 1, ct))
                    fillers.append(mkfiller(k_ct, tq + 1, ct))
            sga = rcpool.tile([97, QT], f32, name=f"sga{tq}", tag="sga", bufs=2)
            sgb = rcpool.tile([97, QT], f32, name=f"sgb{tq}", tag="sgb", bufs=2)
            aos_tiles = []
            allow_late = tq == NQT - 1
            for p in range(4):
                q_ct(tq, p)
                k_ct(tq, p)
                aos_tiles.append(attention(p, tq, sga, sgb, allow_late))
                if p == 1 and pending_np is not None:
                    qt_prev, ao_prev = pending_np[0], normalize(*pending_np)
                    # proj(qt0) drains anywhere; proj(qt1) is reserved for the
                    # last quarter's ACT-bound attention (its own quarter is
                    # PE-bound and only stretches under the extra work);
                    # proj(qt2) is held for the tail, where it keeps the PE
                    # busy+warm under the final reciprocal chain.
                    dest = {0: fillers, 1: late_fillers, 2: held_tail}[qt_prev]
                    for ct in range(NET):
                        dest.append(mkfiller(proj_ct, qt_prev, ct, ao_prev,
                                             qt_prev == 2))
                    pending_np = None
            pending_np = (tq, sga, sgb, aos_tiles)
        # ---- tail: flush fillers, then the last quarter's normalize split
        # around the held proj(qt2) groups so the PE stays warm while the
        # DVE computes the reciprocals; everything then runs at full clock.
        while fillers:
            fillers.pop(0)()
        while late_fillers:
            late_fillers.pop(0)()
        qt_last = pending_np[0]
        # first half of the held proj groups keeps the PE busy while the
        # DVE finishes the last pair's PSUM drain; the recip chain then
        # overlaps the second half, so the broadcast matmuls start with
        # their inputs ready and the PE never idles into a HAM re-throttle
        for fn in held_tail[:4]:
            fn()
        # always-ready warm matmuls, WAR-pinned to the last pair's ava/avb
        # drains: the scheduler hoists the held proj groups into earlier
        # attention dips (their deps are long ready), but these CANNOT be
        # hoisted, so they fill the reciprocal-chain window and the final
        # proj runs at full clock instead of re-throttled half speed
        junka = psum.tile([65, QT], f32, name="junka", tag="ava", bufs=1)
        junkb = psum.tile([65, QT], f32, name="junkb", tag="avb", bufs=1)
        for _ in range(8):
            nc.tensor.matmul(junka[:], warm[:, 0:65], warm[:],
                             start=True, stop=True)
            nc.tensor.matmul(junkb[:], warm[:, 0:65], warm[:],
                             start=True, stop=True)
        rca, rcb = normalize_pre(qt_last, pending_np[1], pending_np[2])
        for fn in held_tail[4:]:
            fn()
        held_tail.clear()
        ao_last = normalize_post(qt_last, rca, rcb, pending_np[3])
        for ct in range(NET):
            proj_ct(qt_last, ct, ao_last, tail=True)

    nc.compile()
    return nc


def _causal_tiles():
    """[128, 128] lower-triangular 0/1 band mask (dq >= dk)."""
    dk = np.arange(P)[:, None]
    dq = np.arange(P)[None, :]
    return np.ascontiguousarray((dq >= dk).astype(np.float32))


def kernel(hidden_state, attention_mask, W_attn, b_attn, W_proj, b_proj):
    global LAST_RESULT
    hs = np.asarray(hidden_state, np.float32)
    am = np.asarray(attention_mask, np.float32).reshape(B, S)
    wa = np.asarray(W_attn, np.float32)
    ba = np.asarray(b_attn, np.float32)
    wpr = np.asarray(W_proj, np.float32)
    bp = np.asarray(b_proj, np.float32)

    has_bv = bool(np.any(ba[2 * E:3 * E] != 0.0))
    key = ("k", has_bv)
    if key not in _CACHE:
        _CACHE[key] = _build(has_bv)
    nc = _CACHE[key]

    bf16 = ml_dtypes.bfloat16
    causal = _causal_tiles().astype(bf16)
    constr = np.ascontiguousarray(
        np.concatenate([causal, np.ones((P, 64), bf16)], axis=1))
    in_maps = []
    for core in range(NC):
        b = core // 2
        c0 = (core % 2) * EL
        constf = np.ascontiguousarray(np.concatenate(
            [
                ba[c0:c0 + EL].reshape(4, P).T,
                ba[E + c0:E + c0 + EL].reshape(4, P).T,
                ba[2 * E + c0:2 * E + c0 + EL].reshape(4, P).T,
                am[b].reshape(NKT, P).T,
            ],
            axis=1,
        ).astype(np.float32))
        in_maps.append({
            "xt": np.ascontiguousarray(hs[b].T).astype(bf16),
            "wq": np.ascontiguousarray(wa[:, c0:c0 + EL]).astype(bf16),
            "wk": np.ascontiguousarray(wa[:, E + c0:E + c0 + EL]).astype(bf16),
            "wv": np.ascontiguousarray(wa[:, 2 * E + c0:2 * E + c0 + EL]).astype(bf16),
            "wp": np.ascontiguousarray(wpr[c0:c0 + EL, :]).astype(bf16),
            "constf": constf,
            "constr": constr,
        })

    from concourse.bass_utils import run_bass_kernel_spmd

    trace = os.environ.get("KERNEL_TRACE", "") == "1"
    res = run_bass_kernel_spmd(nc, in_maps, core_ids=list(range(NC)), trace=trace)
    LAST_RESULT = res

    full = np.empty((B, S, E), np.float32)
    for b in range(B):
        full[b] = res.results[2 * b]["out"].T.astype(np.float32)
        full[b] += res.results[2 * b + 1]["out"].T.astype(np.float32)
        full[b] += bp
    return full
